# revision 20
# baseline (speedup 1.0000x reference)
"""Tensor-parallel GQA attention block for 8 TRN2 NeuronCores.

Sharding: TP over heads. Core c owns query heads 4c..4c+3 and KV head c
(column-shard of wq/wk/wv), plus the matching column-shard of wo. x is
replicated. Each core computes its partial x@wo_shard.T contribution and a
ReduceScatter sums them, leaving each core with a row-shard of the output;
the host concatenates the shards.

All device compute in bf16 with f32 PSUM accumulation. Host-side input prep:
weight shards are transposed to contraction-major layout, query-head dims are
permuted to (evens, odds) so RoPE halves sit in partition halves.

Scores are computed transposed (S^T = K @ Q^T, [k, q] layout) so the exp'd
probabilities feed the P@V matmul directly as the moving operand; the
1/sqrt(HD) scale is applied inside the exp activation. Softmax denominators
are partition-reduced on GpSimd and applied as a broadcast divide. exp() runs
without max-subtraction: scores here are bounded (|score| < ~15), safe in f32.
"""
import os

import numpy as np
import ml_dtypes

import concourse.mybir as mybir
import concourse.tile as tile
from concourse import bacc
import concourse.bass_utils as _bu
from concourse.bass_utils import run_bass_kernel_spmd
from concourse.masks import make_identity

if os.environ.get("BASS_LDW_OPT") and not getattr(_bu, "_ldw_patched", False):
    _orig_run_command = _bu.run_command

    def _run_command_ldw(argv, **kw):
        argv = ["--enable-ldw-opt=true" if a == "--enable-ldw-opt=false"
                else a for a in argv]
        return _orig_run_command(argv, **kw)

    _bu.run_command = _run_command_ldw
    _bu._ldw_patched = True

N_CORES = 8
B, S, D = 1, 2048, 4096
H, KVH, HD = 32, 8, 128
HL = H // N_CORES          # 4 local q heads
SCALE = HD ** -0.5
P = 128
QC = 512                   # attention q-chunk width
NQC = S // QC              # 4
NKB = S // P               # 16 k-tiles
NDC = D // P               # 32 D-chunks
PW = 512                   # phase-1 s-panel width
NPAN = S // PW             # 4
SHARD = QC // N_CORES      # 64 rows per core per RS chunk

FDT = mybir.dt.float32
BDT = mybir.dt.bfloat16
HDT = mybir.dt.float16
NEG = -1.0e9

LAST_RESULT = None


def _build(mode):
    nc = bacc.Bacc("TRN2", target_bir_lowering=False, debug=False,
                   num_devices=N_CORES)
    x_ext = nc.dram_tensor("x", [S, D], FDT, kind="ExternalInput")
    wqt_ext = nc.dram_tensor("wqt", [D, HL * P], BDT, kind="ExternalInput")
    wkt_ext = nc.dram_tensor("wkt", [D, P], BDT, kind="ExternalInput")
    wvt_ext = nc.dram_tensor("wvt", [D, P], BDT, kind="ExternalInput")
    wot_ext = nc.dram_tensor("wot", [HL * P, D], BDT, kind="ExternalInput")
    c2_ext = nc.dram_tensor("c2", [P, S], BDT, kind="ExternalInput")
    s2_ext = nc.dram_tensor("s2", [P, S], BDT, kind="ExternalInput")
    if mode == "causal":
        tri_ext = nc.dram_tensor("tri", [P, 896], FDT, kind="ExternalInput")
    if mode == "mask":
        maskt_ext = nc.dram_tensor("maskt", [S, S], FDT, kind="ExternalInput")
    out_ext = nc.dram_tensor("out", [NQC * SHARD, D], HDT,
                             kind="ExternalOutput")

    Alu = mybir.AluOpType
    Act = mybir.ActivationFunctionType

    with tile.TileContext(nc) as tc:
        with tc.tile_pool(name="persist", bufs=1) as pers:
            qt_sb = pers.tile([P, HL * S], BDT, tag="qt")     # [hd,(h,s)]
            ones_sb = pers.tile([P, P], BDT, tag="ones")
            nc.vector.memset(ones_sb[:], 1.0)
            ident = pers.tile([P, P], BDT, tag="ident")
            make_identity(nc, ident[:])
            kt_sb = pers.tile([P, S], BDT, tag="kt")
            v_sb = pers.tile([P, NKB * P], BDT, tag="v")      # [s,(kb,hd)]
            if mode == "causal":
                tri_sb = pers.tile([P, 896], FDT, tag="tri")
                nc.sync.dma_start(tri_sb[:], tri_ext[:])

            # ---------------- phase 1: QKV projections + RoPE -------------
            with (
                tc.tile_pool(name="ph1", bufs=1) as ph1,
                tc.tile_pool(name="xstage", bufs=2) as xst,
                tc.tile_pool(name="rsc", bufs=2) as rsc,
                tc.tile_pool(name="ppsum", bufs=1, space="PSUM") as ppsum,
                tc.tile_pool(name="tpsum", bufs=2, space="PSUM") as tpsum,
            ):
                wqt_sb = ph1.tile([P, NDC * HL * P], BDT, tag="wqt")
                nc.sync.dma_start(
                    wqt_sb[:].rearrange("p (dc h) -> p dc h", dc=NDC),
                    wqt_ext[:].rearrange("(dc p) h -> p dc h", p=P),
                )
                wkt_sb = ph1.tile([P, NDC * P], BDT, tag="wkt")
                nc.sync.dma_start(
                    wkt_sb[:].rearrange("p (dc h) -> p dc h", dc=NDC),
                    wkt_ext[:].rearrange("(dc p) h -> p dc h", p=P),
                )
                wvt_sb = ph1.tile([P, NDC * P], BDT, tag="wvt")
                nc.sync.dma_start(
                    wvt_sb[:].rearrange("p (dc h) -> p dc h", dc=NDC),
                    wvt_ext[:].rearrange("(dc p) h -> p dc h", p=P),
                )
                c2_sb = ph1.tile([P, S], BDT, tag="c2")
                nc.sync.dma_start(c2_sb[:], c2_ext[:])
                s2_sb = ph1.tile([P, S], BDT, tag="s2")
                nc.sync.dma_start(s2_sb[:], s2_ext[:])

                def rope(acc, c2, s2, out):
                    """acc: [128,PW] f32 PSUM (pre-RoPE, (evens,odds) perm).
                    out = acc*c2 + swap_halves(acc)*s2 (signs folded in s2).
                    """
                    t_sb = rsc.tile([P, PW], BDT, tag="ropet")
                    nc.scalar.copy(t_sb[:], acc[:])
                    tsw = rsc.tile([P, PW], BDT, tag="ropesw")
                    nc.sync.dma_start(tsw[0:64, :], t_sb[64:128, :])
                    nc.sync.dma_start(tsw[64:128, :], t_sb[0:64, :])
                    m = rsc.tile([P, PW], FDT, tag="ropem")
                    nc.vector.tensor_tensor(m[:], acc[:], c2, op=Alu.mult)
                    n = rsc.tile([P, PW], FDT, tag="ropen")
                    nc.vector.tensor_tensor(n[:], tsw[:], s2, op=Alu.mult)
                    nc.vector.tensor_tensor(out, m[:], n[:], op=Alu.add)

                def rope_panel(pan, qacc, kacc, vacc):
                    cols = slice(pan * PW, (pan + 1) * PW)
                    for h in range(HL):
                        rope(qacc[h], c2_sb[:, cols], s2_sb[:, cols],
                             qt_sb[:, h * S + pan * PW: h * S + (pan + 1) * PW])
                    rope(kacc, c2_sb[:, cols], s2_sb[:, cols],
                         kt_sb[:, cols])
                    vtmp = rsc.tile([P, PW], BDT, tag="vtmp")
                    nc.scalar.copy(vtmp[:], vacc[:])
                    vtp = tpsum.tile([P, PW], BDT, tag="xtp", name="vtp")
                    for st4 in range(4):
                        nc.tensor.transpose(
                            vtp[:, st4 * P:(st4 + 1) * P],
                            vtmp[:, st4 * P:(st4 + 1) * P], ident[:])
                    nc.scalar.copy(
                        v_sb[:, pan * PW:(pan + 1) * PW], vtp[:])

                prev = None
                for pan in range(NPAN):
                    xbfs = []
                    for st4 in range(4):
                        xf = xst.tile([P, D], FDT, tag="xf32")
                        r0 = (pan * 4 + st4) * P
                        nc.sync.dma_start(xf[:], x_ext[r0:r0 + P, :])
                        xb = xst.tile([P, D], BDT, tag="xbf", bufs=6,
                                      name="xb")
                        nc.vector.tensor_copy(out=xb[:], in_=xf[:])
                        xbfs.append(xb)
                    if prev is not None:
                        rope_panel(*prev)
                    qacc = [ppsum.tile([P, PW], FDT, tag=f"qacc{h}",
                                       name=f"qacc{h}")
                            for h in range(HL)]
                    kacc = ppsum.tile([P, PW], FDT, tag="kacc")
                    vacc = ppsum.tile([P, PW], FDT, tag="vacc")
                    for dc in range(NDC):
                        xtp = tpsum.tile([P, PW], BDT, tag="xtp")
                        for st4 in range(4):
                            nc.tensor.transpose(
                                xtp[:, st4 * P:(st4 + 1) * P],
                                xbfs[st4][:, dc * P:dc * P + P],
                                ident[:],
                            )
                        xts = xst.tile([P, PW], BDT, tag="xts", bufs=3)
                        nc.scalar.copy(xts[:], xtp[:])
                        first, last = dc == 0, dc == NDC - 1
                        for h in range(HL):
                            nc.tensor.matmul(
                                qacc[h][:],
                                wqt_sb[:, dc * HL * P + h * P:
                                       dc * HL * P + (h + 1) * P],
                                xts[:], start=first, stop=last)
                        nc.tensor.matmul(
                            kacc[:], wkt_sb[:, dc * P:(dc + 1) * P],
                            xts[:], start=first, stop=last)
                        nc.tensor.matmul(
                            vacc[:], wvt_sb[:, dc * P:(dc + 1) * P],
                            xts[:], start=first, stop=last)
                    prev = (pan, qacc, kacc, vacc)
                rope_panel(*prev)

            # ---------------- phase 2: attention + wo + ReduceScatter -----
            with (
                tc.tile_pool(name="ph2", bufs=1) as ph2,
                tc.tile_pool(name="att", bufs=2) as att,
                tc.tile_pool(name="ptpool", bufs=4) as ptp,
                tc.tile_pool(name="scr", bufs=2) as scp,
                tc.tile_pool(name="mtpool", bufs=17) as mtp,
                tc.tile_pool(name="cdram", bufs=2, space="DRAM") as cdram,
                tc.tile_pool(name="stps", bufs=2, space="PSUM") as stps,
                tc.tile_pool(name="avps", bufs=2, space="PSUM") as avps,
                tc.tile_pool(name="wops", bufs=2, space="PSUM") as wops,
            ):
                wot_sb = ph2.tile([P, HL * D], BDT, tag="wot")  # [hc,(hc,do)]
                nc.sync.dma_start(
                    wot_sb[:].rearrange("p (hc d) -> p hc d", hc=HL),
                    wot_ext[:].rearrange("(hc p) d -> p hc d", p=P),
                )
                pending = []

                def flush_rs():
                    while pending:
                        out_off, nrows, t = pending.pop(0)
                        nr = nrows // N_CORES
                        rs_out = cdram.tile([nr, D], HDT,
                                            tag="rsout", name="rs_out")
                        nc.gpsimd.collective_compute(
                            "ReduceScatter", Alu.add,
                            ins=[t[:]], outs=[rs_out[:]],
                            replica_groups=[list(range(N_CORES))])
                        nc.gpsimd.dma_start(
                            out_ext[out_off:out_off + nr, :], rs_out[:])

                out_off = 0
                for qc in range(NQC):
                    nkb = 4 * qc + 4 if mode == "causal" else NKB
                    mts = []
                    if mode == "mask":
                        for kb in range(nkb):
                            mt = mtp.tile([P, QC], FDT, tag="mt", name="mt")
                            nc.sync.dma_start(
                                mt[:],
                                maskt_ext[kb * P:(kb + 1) * P,
                                          qc * QC:(qc + 1) * QC])
                            mts.append(mt)
                    attn_t = []
                    for h in range(HL):
                        ssum = avps.tile([P, QC], FDT, tag="ssum")
                        avt = avps.tile([P, QC], FDT, tag="avt")
                        q0 = h * S + qc * QC
                        for kb in range(nkb):
                            # causal: columns below c0 are fully masked
                            c0 = (kb - 4 * qc) * P \
                                if mode == "causal" and kb >= 4 * qc else 0
                            st_ps = stps.tile([P, QC], FDT, tag="st")
                            nc.tensor.matmul(
                                st_ps[:, c0:QC],
                                kt_sb[:, kb * P:(kb + 1) * P],
                                qt_sb[:, q0 + c0:q0 + QC],
                                start=True, stop=True)
                            if mode == "causal" and kb >= 4 * qc:
                                nc.vector.tensor_tensor(
                                    st_ps[:, c0:c0 + P], st_ps[:, c0:c0 + P],
                                    tri_sb[:, 384:384 + P],
                                    op=Alu.add)
                            elif mode == "mask":
                                nc.vector.tensor_tensor(
                                    st_ps[:], st_ps[:], mts[kb][:],
                                    op=Alu.add)
                            pt = ptp.tile([P, QC], BDT, tag="pt")
                            nc.scalar.activation(pt[:, c0:QC],
                                                 st_ps[:, c0:QC], Act.Exp,
                                                 scale=float(SCALE))
                            first, last = kb == 0, kb == nkb - 1
                            nc.tensor.matmul(
                                ssum[:, c0:QC], ones_sb[:], pt[:, c0:QC],
                                start=first, stop=last)
                            nc.tensor.matmul(
                                avt[:, c0:QC],
                                v_sb[:, kb * P:(kb + 1) * P], pt[:, c0:QC],
                                start=first, stop=last)
                        rsb = scp.tile([P, QC], FDT, tag="rsb")
                        nc.vector.reciprocal_approx_fast(out=rsb[:],
                                                         in_=ssum[:])
                        at = att.tile([P, QC], BDT, tag=f"attnT{h}",
                                      name=f"attnT{h}")
                        nc.vector.tensor_tensor(at[:], avt[:], rsb[:],
                                                op=Alu.mult)
                        attn_t.append(at)
                    flush_rs()
                    halves = 4 if qc == NQC - 1 else 1
                    for half in range(halves):
                        hrows = QC // halves
                        rs_in = cdram.tile([hrows, D], HDT, tag="rsin",
                                           bufs=2, name="rs_in",
                                           padded_shape=[QC, D])
                        for st2 in range(4 // halves):
                            st4 = half * (4 // halves) + st2  # noqa
                            for do in range(8):
                                ops = wops.tile([P, QC], FDT, tag="wops")
                                for hc in range(HL):
                                    nc.tensor.matmul(
                                        ops[:],
                                        attn_t[hc][:, st4 * P:(st4 + 1) * P],
                                        wot_sb[:, hc * D + do * QC:
                                               hc * D + (do + 1) * QC],
                                        start=(hc == 0), stop=(hc == HL - 1))
                                rs_sb = scp.tile([P, QC], HDT, tag="rssb")
                                nc.scalar.copy(rs_sb[:], ops[:])
                                nc.sync.dma_start(
                                    rs_in[st2 * P:(st2 + 1) * P,
                                          do * QC:(do + 1) * QC], rs_sb[:])
                        pending.append((out_off, hrows, rs_in))
                        out_off += hrows // N_CORES
                flush_rs()
    nc.compile()
    return nc


def _prep_inputs(x, freqs_cos, freqs_sin, mask, wq, wk, wv, wo, mode):
    bf16 = ml_dtypes.bfloat16
    perm = np.concatenate([np.arange(0, HD, 2), np.arange(1, HD, 2)])
    x2 = np.ascontiguousarray(x.reshape(S, D), dtype=np.float32)
    cosT = np.ascontiguousarray(freqs_cos.T, dtype=np.float32)  # (64, S)
    sinT = np.ascontiguousarray(freqs_sin.T, dtype=np.float32)
    c2 = np.ascontiguousarray(np.vstack([cosT, cosT]).astype(bf16))
    s2 = np.ascontiguousarray(np.vstack([-sinT, sinT]).astype(bf16))
    t = np.arange(896) - 384
    tri = np.where(t[None, :] >= np.arange(P)[:, None], 0.0,
                   NEG / SCALE).astype(np.float32)
    wq4 = wq.reshape(H, HD, D)[:, perm, :]
    wk4 = wk.reshape(KVH, HD, D)[:, perm, :]
    wv4 = wv.reshape(KVH, HD, D)
    in_maps = []
    for c in range(N_CORES):
        wqs = wq4[c * HL:(c + 1) * HL].reshape(HL * HD, D)
        m = {
            "x": x2,
            "wqt": np.ascontiguousarray(wqs.T).astype(bf16),
            "wkt": np.ascontiguousarray(wk4[c].T).astype(bf16),
            "wvt": np.ascontiguousarray(wv4[c].T).astype(bf16),
            "wot": np.ascontiguousarray(
                wo[:, c * HL * HD:(c + 1) * HL * HD].T).astype(bf16),
            "c2": c2, "s2": s2,
        }
        if mode == "causal":
            m["tri"] = tri
        if mode == "mask":
            m["maskt"] = np.ascontiguousarray(
                mask.T / SCALE, dtype=np.float32)
        in_maps.append(m)
    return in_maps


def _mask_mode(mask):
    if np.all(mask == 0):
        return "zeros"
    iu = np.triu_indices(S, 1)
    if (np.all(np.tril(mask) == 0) and np.all(mask[iu] <= -1e8)
            and np.all(mask[iu] >= -2e9)):
        return "causal"
    return "mask"


_GRAPH_CACHE = {}


def kernel(x, freqs_cos, freqs_sin, mask, wq, wk, wv, wo):
    global LAST_RESULT
    mode = _mask_mode(np.asarray(mask))
    if mode not in _GRAPH_CACHE:
        _GRAPH_CACHE[mode] = _build(mode)
    nc = _GRAPH_CACHE[mode]
    in_maps = _prep_inputs(
        np.asarray(x), np.asarray(freqs_cos), np.asarray(freqs_sin),
        np.asarray(mask), np.asarray(wq), np.asarray(wk), np.asarray(wv),
        np.asarray(wo), mode)
    res = run_bass_kernel_spmd(
        nc, in_maps, core_ids=list(range(N_CORES)),
        trace=bool(os.environ.get("BASS_TRACE")))
    LAST_RESULT = res
    out = np.empty((S, D), dtype=np.float32)
    chunks = [(0, 512), (512, 512), (1024, 512),
              (1536, 128), (1664, 128), (1792, 128), (1920, 128)]
    for c in range(N_CORES):
        shard = np.asarray(res.results[c]["out"], dtype=np.float32)
        off = 0
        for src_row0, nrows in chunks:
            nr = nrows // N_CORES
            out[src_row0 + c * nr: src_row0 + (c + 1) * nr] = \
                shard[off:off + nr]
            off += nr
    return out.reshape(B, S, D)


# revision 21
# speedup vs baseline: 1.0209x; 1.0209x over previous
"""Tensor-parallel GQA attention block for 8 TRN2 NeuronCores.

Sharding: TP over heads. Core c owns query heads 4c..4c+3 and KV head c
(column-shard of wq/wk/wv), plus the matching column-shard of wo. x is
replicated. Each core computes its partial x@wo_shard.T contribution and a
ReduceScatter sums them, leaving each core with a row-shard of the output;
the host concatenates the shards.

All device compute in bf16 with f32 PSUM accumulation. Host-side input prep:
weight shards are transposed to contraction-major layout, query-head dims are
permuted to (evens, odds) so RoPE halves sit in partition halves.

Scores are computed transposed (S^T = K @ Q^T, [k, q] layout) so the exp'd
probabilities feed the P@V matmul directly as the moving operand; the
1/sqrt(HD) scale is applied inside the exp activation. Softmax denominators
are partition-reduced on GpSimd and applied as a broadcast divide. exp() runs
without max-subtraction: scores here are bounded (|score| < ~15), safe in f32.
"""
import os

import numpy as np
import ml_dtypes

import concourse.mybir as mybir
import concourse.tile as tile
from concourse import bacc
import concourse.bass_utils as _bu
from concourse.bass_utils import run_bass_kernel_spmd
from concourse.masks import make_identity

if os.environ.get("BASS_LDW_OPT") and not getattr(_bu, "_ldw_patched", False):
    _orig_run_command = _bu.run_command

    def _run_command_ldw(argv, **kw):
        argv = ["--enable-ldw-opt=true" if a == "--enable-ldw-opt=false"
                else a for a in argv]
        return _orig_run_command(argv, **kw)

    _bu.run_command = _run_command_ldw
    _bu._ldw_patched = True

N_CORES = 8
B, S, D = 1, 2048, 4096
H, KVH, HD = 32, 8, 128
HL = H // N_CORES          # 4 local q heads
SCALE = HD ** -0.5
P = 128
QC = 512                   # attention q-chunk width
NQC = S // QC              # 4
NKB = S // P               # 16 k-tiles
NDC = D // P               # 32 D-chunks
PW = 512                   # phase-1 s-panel width
NPAN = S // PW             # 4
SHARD = QC // N_CORES      # 64 rows per core per RS chunk

FDT = mybir.dt.float32
BDT = mybir.dt.bfloat16
HDT = mybir.dt.float16
NEG = -1.0e9

LAST_RESULT = None


def _build(mode):
    nc = bacc.Bacc("TRN2", target_bir_lowering=False, debug=False,
                   num_devices=N_CORES)
    x_ext = nc.dram_tensor("x", [S, D], FDT, kind="ExternalInput")
    wqt_ext = nc.dram_tensor("wqt", [D, HL * P], BDT, kind="ExternalInput")
    wkt_ext = nc.dram_tensor("wkt", [D, P], BDT, kind="ExternalInput")
    wvt_ext = nc.dram_tensor("wvt", [D, P], BDT, kind="ExternalInput")
    wot_ext = nc.dram_tensor("wot", [HL * P, D], BDT, kind="ExternalInput")
    c2_ext = nc.dram_tensor("c2", [P, S], BDT, kind="ExternalInput")
    s2_ext = nc.dram_tensor("s2", [P, S], BDT, kind="ExternalInput")
    if mode == "causal":
        tri_ext = nc.dram_tensor("tri", [P, 896], FDT, kind="ExternalInput")
    if mode == "mask":
        maskt_ext = nc.dram_tensor("maskt", [S, S], FDT, kind="ExternalInput")
    out_ext = nc.dram_tensor("out", [NQC * SHARD, D], HDT,
                             kind="ExternalOutput")

    Alu = mybir.AluOpType
    Act = mybir.ActivationFunctionType

    with tile.TileContext(nc) as tc:
        with tc.tile_pool(name="persist", bufs=1) as pers:
            qt_sb = pers.tile([P, HL * S], BDT, tag="qt")     # [hd,(h,s)]
            ones_sb = pers.tile([P, P], BDT, tag="ones")
            nc.vector.memset(ones_sb[:], 1.0)
            ident = pers.tile([P, P], BDT, tag="ident")
            make_identity(nc, ident[:])
            kt_sb = pers.tile([P, S], BDT, tag="kt")
            v_sb = pers.tile([P, NKB * P], BDT, tag="v")      # [s,(kb,hd)]
            if mode == "causal":
                tri_sb = pers.tile([P, 896], FDT, tag="tri")
                nc.sync.dma_start(tri_sb[:], tri_ext[:])

            # ---------------- phase 1: QKV projections + RoPE -------------
            with (
                tc.tile_pool(name="ph1", bufs=1) as ph1,
                tc.tile_pool(name="xstage", bufs=2) as xst,
                tc.tile_pool(name="rsc", bufs=2) as rsc,
                tc.tile_pool(name="ppsum", bufs=1, space="PSUM") as ppsum,
                tc.tile_pool(name="tpsum", bufs=2, space="PSUM") as tpsum,
            ):
                wqt_sb = ph1.tile([P, NDC * HL * P], BDT, tag="wqt")
                nc.sync.dma_start(
                    wqt_sb[:].rearrange("p (dc h) -> p dc h", dc=NDC),
                    wqt_ext[:].rearrange("(dc p) h -> p dc h", p=P),
                )
                wkt_sb = ph1.tile([P, NDC * P], BDT, tag="wkt")
                nc.sync.dma_start(
                    wkt_sb[:].rearrange("p (dc h) -> p dc h", dc=NDC),
                    wkt_ext[:].rearrange("(dc p) h -> p dc h", p=P),
                )
                wvt_sb = ph1.tile([P, NDC * P], BDT, tag="wvt")
                nc.sync.dma_start(
                    wvt_sb[:].rearrange("p (dc h) -> p dc h", dc=NDC),
                    wvt_ext[:].rearrange("(dc p) h -> p dc h", p=P),
                )
                c2_sb = ph1.tile([P, S], BDT, tag="c2")
                nc.sync.dma_start(c2_sb[:], c2_ext[:])
                s2_sb = ph1.tile([P, S], BDT, tag="s2")
                nc.sync.dma_start(s2_sb[:], s2_ext[:])

                def rope(acc, c2, s2, out):
                    """acc: [128,PW] f32 PSUM (pre-RoPE, (evens,odds) perm).
                    out = acc*c2 + swap_halves(acc)*s2 (signs folded in s2).
                    """
                    t_sb = rsc.tile([P, PW], BDT, tag="ropet")
                    nc.scalar.copy(t_sb[:], acc[:])
                    tsw = rsc.tile([P, PW], BDT, tag="ropesw")
                    nc.sync.dma_start(tsw[0:64, :], t_sb[64:128, :])
                    nc.sync.dma_start(tsw[64:128, :], t_sb[0:64, :])
                    m = rsc.tile([P, PW], FDT, tag="ropem")
                    nc.vector.tensor_tensor(m[:], acc[:], c2, op=Alu.mult)
                    n = rsc.tile([P, PW], FDT, tag="ropen")
                    nc.vector.tensor_tensor(n[:], tsw[:], s2, op=Alu.mult)
                    nc.vector.tensor_tensor(out, m[:], n[:], op=Alu.add)

                def rope_panel(pan, qacc, kacc, vacc):
                    cols = slice(pan * PW, (pan + 1) * PW)
                    for h in range(HL):
                        rope(qacc[h], c2_sb[:, cols], s2_sb[:, cols],
                             qt_sb[:, h * S + pan * PW: h * S + (pan + 1) * PW])
                    rope(kacc, c2_sb[:, cols], s2_sb[:, cols],
                         kt_sb[:, cols])
                    vtmp = rsc.tile([P, PW], BDT, tag="vtmp")
                    nc.scalar.copy(vtmp[:], vacc[:])
                    vtp = tpsum.tile([P, PW], BDT, tag="xtp", name="vtp")
                    for st4 in range(4):
                        nc.tensor.transpose(
                            vtp[:, st4 * P:(st4 + 1) * P],
                            vtmp[:, st4 * P:(st4 + 1) * P], ident[:])
                    nc.scalar.copy(
                        v_sb[:, pan * PW:(pan + 1) * PW], vtp[:])

                prev = None
                for pan in range(NPAN):
                    xbfs = []
                    for st4 in range(4):
                        xf = xst.tile([P, D], FDT, tag="xf32")
                        r0 = (pan * 4 + st4) * P
                        nc.sync.dma_start(xf[:], x_ext[r0:r0 + P, :])
                        xb = xst.tile([P, D], BDT, tag="xbf", bufs=6,
                                      name="xb")
                        nc.vector.tensor_copy(out=xb[:], in_=xf[:])
                        xbfs.append(xb)
                    if prev is not None:
                        rope_panel(*prev)
                    qacc = [ppsum.tile([P, PW], FDT, tag=f"qacc{h}",
                                       name=f"qacc{h}")
                            for h in range(HL)]
                    kacc = ppsum.tile([P, PW], FDT, tag="kacc")
                    vacc = ppsum.tile([P, PW], FDT, tag="vacc")
                    for dc in range(NDC):
                        xtp = tpsum.tile([P, PW], BDT, tag="xtp")
                        for st4 in range(4):
                            nc.tensor.transpose(
                                xtp[:, st4 * P:(st4 + 1) * P],
                                xbfs[st4][:, dc * P:dc * P + P],
                                ident[:],
                            )
                        xts = xst.tile([P, PW], BDT, tag="xts", bufs=3)
                        nc.scalar.copy(xts[:], xtp[:])
                        first, last = dc == 0, dc == NDC - 1
                        for h in range(HL):
                            nc.tensor.matmul(
                                qacc[h][:],
                                wqt_sb[:, dc * HL * P + h * P:
                                       dc * HL * P + (h + 1) * P],
                                xts[:], start=first, stop=last)
                        nc.tensor.matmul(
                            kacc[:], wkt_sb[:, dc * P:(dc + 1) * P],
                            xts[:], start=first, stop=last)
                        nc.tensor.matmul(
                            vacc[:], wvt_sb[:, dc * P:(dc + 1) * P],
                            xts[:], start=first, stop=last)
                    prev = (pan, qacc, kacc, vacc)
                rope_panel(*prev)

            # ---------------- phase 2: attention + wo + ReduceScatter -----
            with (
                tc.tile_pool(name="ph2", bufs=1) as ph2,
                tc.tile_pool(name="att", bufs=2) as att,
                tc.tile_pool(name="ptpool", bufs=4) as ptp,
                tc.tile_pool(name="scr", bufs=2) as scp,
                tc.tile_pool(name="mtpool", bufs=17) as mtp,
                tc.tile_pool(name="cdram", bufs=2, space="DRAM") as cdram,
                tc.tile_pool(name="stps", bufs=2, space="PSUM") as stps,
                tc.tile_pool(name="avps", bufs=2, space="PSUM") as avps,
                tc.tile_pool(name="wops", bufs=2, space="PSUM") as wops,
            ):
                wot_sb = ph2.tile([P, HL * D], BDT, tag="wot")  # [hc,(hc,do)]
                nc.sync.dma_start(
                    wot_sb[:].rearrange("p (hc d) -> p hc d", hc=HL),
                    wot_ext[:].rearrange("(hc p) d -> p hc d", p=P),
                )
                pending = []

                def flush_rs():
                    while pending:
                        out_off, nrows, t = pending.pop(0)
                        nr = nrows // N_CORES
                        rs_out = cdram.tile([nr, D], HDT,
                                            tag="rsout", name="rs_out")
                        nc.gpsimd.collective_compute(
                            "ReduceScatter", Alu.add,
                            ins=[t[:]], outs=[rs_out[:]],
                            replica_groups=[list(range(N_CORES))])
                        nc.gpsimd.dma_start(
                            out_ext[out_off:out_off + nr, :], rs_out[:])

                out_off = 0
                for qc in range(NQC):
                    nkb = 4 * qc + 4 if mode == "causal" else NKB
                    mts = []
                    if mode == "mask":
                        for kb in range(nkb):
                            mt = mtp.tile([P, QC], FDT, tag="mt", name="mt")
                            nc.sync.dma_start(
                                mt[:],
                                maskt_ext[kb * P:(kb + 1) * P,
                                          qc * QC:(qc + 1) * QC])
                            mts.append(mt)
                    attn_t = []
                    for h in range(HL):
                        ssum = avps.tile([P, QC], FDT, tag="ssum")
                        avt = avps.tile([P, QC], FDT, tag="avt")
                        q0 = h * S + qc * QC
                        for kb in range(nkb):
                            # causal: columns below c0 are fully masked
                            c0 = (kb - 4 * qc) * P \
                                if mode == "causal" and kb >= 4 * qc else 0
                            st_ps = stps.tile([P, QC], FDT, tag="st")
                            nc.tensor.matmul(
                                st_ps[:, c0:QC],
                                kt_sb[:, kb * P:(kb + 1) * P],
                                qt_sb[:, q0 + c0:q0 + QC],
                                start=True, stop=True)
                            if mode == "causal" and kb >= 4 * qc:
                                nc.vector.tensor_tensor(
                                    st_ps[:, c0:c0 + P], st_ps[:, c0:c0 + P],
                                    tri_sb[:, 384:384 + P],
                                    op=Alu.add)
                            elif mode == "mask":
                                nc.vector.tensor_tensor(
                                    st_ps[:], st_ps[:], mts[kb][:],
                                    op=Alu.add)
                            pt = ptp.tile([P, QC], BDT, tag="pt")
                            nc.scalar.activation(pt[:, c0:QC],
                                                 st_ps[:, c0:QC], Act.Exp,
                                                 scale=float(SCALE))
                            first, last = kb == 0, kb == nkb - 1
                            nc.tensor.matmul(
                                ssum[:, c0:QC], ones_sb[:], pt[:, c0:QC],
                                start=first, stop=last)
                            nc.tensor.matmul(
                                avt[:, c0:QC],
                                v_sb[:, kb * P:(kb + 1) * P], pt[:, c0:QC],
                                start=first, stop=last)
                        rsb = scp.tile([P, QC], FDT, tag="rsb")
                        nc.vector.reciprocal_approx_fast(out=rsb[:],
                                                         in_=ssum[:])
                        at = att.tile([P, QC], BDT, tag=f"attnT{h}",
                                      name=f"attnT{h}")
                        nc.vector.tensor_tensor(at[:], avt[:], rsb[:],
                                                op=Alu.mult)
                        attn_t.append(at)
                    flush_rs()
                    halves = 2 if qc == NQC - 1 else 1
                    for half in range(halves):
                        hrows = QC // halves
                        rs_in = cdram.tile([hrows, D], HDT, tag="rsin",
                                           bufs=2, name="rs_in",
                                           padded_shape=[QC, D])
                        for st2 in range(4 // halves):
                            st4 = half * (4 // halves) + st2  # noqa
                            for do in range(8):
                                ops = wops.tile([P, QC], FDT, tag="wops")
                                for hc in range(HL):
                                    nc.tensor.matmul(
                                        ops[:],
                                        attn_t[hc][:, st4 * P:(st4 + 1) * P],
                                        wot_sb[:, hc * D + do * QC:
                                               hc * D + (do + 1) * QC],
                                        start=(hc == 0), stop=(hc == HL - 1))
                                rs_sb = scp.tile([P, QC], HDT, tag="rssb")
                                nc.scalar.copy(rs_sb[:], ops[:])
                                nc.sync.dma_start(
                                    rs_in[st2 * P:(st2 + 1) * P,
                                          do * QC:(do + 1) * QC], rs_sb[:])
                        pending.append((out_off, hrows, rs_in))
                        out_off += hrows // N_CORES
                flush_rs()
    nc.compile()
    return nc


def _prep_inputs(x, freqs_cos, freqs_sin, mask, wq, wk, wv, wo, mode):
    bf16 = ml_dtypes.bfloat16
    perm = np.concatenate([np.arange(0, HD, 2), np.arange(1, HD, 2)])
    x2 = np.ascontiguousarray(x.reshape(S, D), dtype=np.float32)
    cosT = np.ascontiguousarray(freqs_cos.T, dtype=np.float32)  # (64, S)
    sinT = np.ascontiguousarray(freqs_sin.T, dtype=np.float32)
    c2 = np.ascontiguousarray(np.vstack([cosT, cosT]).astype(bf16))
    s2 = np.ascontiguousarray(np.vstack([-sinT, sinT]).astype(bf16))
    t = np.arange(896) - 384
    tri = np.where(t[None, :] >= np.arange(P)[:, None], 0.0,
                   NEG / SCALE).astype(np.float32)
    wq4 = wq.reshape(H, HD, D)[:, perm, :]
    wk4 = wk.reshape(KVH, HD, D)[:, perm, :]
    wv4 = wv.reshape(KVH, HD, D)
    in_maps = []
    for c in range(N_CORES):
        wqs = wq4[c * HL:(c + 1) * HL].reshape(HL * HD, D)
        m = {
            "x": x2,
            "wqt": np.ascontiguousarray(wqs.T).astype(bf16),
            "wkt": np.ascontiguousarray(wk4[c].T).astype(bf16),
            "wvt": np.ascontiguousarray(wv4[c].T).astype(bf16),
            "wot": np.ascontiguousarray(
                wo[:, c * HL * HD:(c + 1) * HL * HD].T).astype(bf16),
            "c2": c2, "s2": s2,
        }
        if mode == "causal":
            m["tri"] = tri
        if mode == "mask":
            m["maskt"] = np.ascontiguousarray(
                mask.T / SCALE, dtype=np.float32)
        in_maps.append(m)
    return in_maps


def _mask_mode(mask):
    if np.all(mask == 0):
        return "zeros"
    iu = np.triu_indices(S, 1)
    if (np.all(np.tril(mask) == 0) and np.all(mask[iu] <= -1e8)
            and np.all(mask[iu] >= -2e9)):
        return "causal"
    return "mask"


_GRAPH_CACHE = {}


def kernel(x, freqs_cos, freqs_sin, mask, wq, wk, wv, wo):
    global LAST_RESULT
    mode = _mask_mode(np.asarray(mask))
    if mode not in _GRAPH_CACHE:
        _GRAPH_CACHE[mode] = _build(mode)
    nc = _GRAPH_CACHE[mode]
    in_maps = _prep_inputs(
        np.asarray(x), np.asarray(freqs_cos), np.asarray(freqs_sin),
        np.asarray(mask), np.asarray(wq), np.asarray(wk), np.asarray(wv),
        np.asarray(wo), mode)
    res = run_bass_kernel_spmd(
        nc, in_maps, core_ids=list(range(N_CORES)),
        trace=bool(os.environ.get("BASS_TRACE")))
    LAST_RESULT = res
    out = np.empty((S, D), dtype=np.float32)
    chunks = [(0, 512), (512, 512), (1024, 512), (1536, 256), (1792, 256)]
    for c in range(N_CORES):
        shard = np.asarray(res.results[c]["out"], dtype=np.float32)
        off = 0
        for src_row0, nrows in chunks:
            nr = nrows // N_CORES
            out[src_row0 + c * nr: src_row0 + (c + 1) * nr] = \
                shard[off:off + nr]
            off += nr
    return out.reshape(B, S, D)
